# revision 1
# baseline (speedup 1.0000x reference)
"""CoaT factorized-attention block kernel for Trainium2, 8 NeuronCores.

Sharding: data-parallel over batch B=8 -> one batch element per core.

Per-core pipeline (all attention-path tensors in head-aligned [96, *] layout):
  P1  (per 128-token chunk): k,v = x @ Wk/Wv (natural layout, PE);
      E = exp(k) (ACT); kv_aug[h] += E_h^T @ [v_h | 1] (PE, PSUM-resident
      accumulator across chunks); v^T via PE transpose -> vt[h] [96, 3136].
  P2  kv[h] = kv_aug[:, :96] * scale / kv_aug[:, 96]  (DVE).
  P3  cls column: q_cls, factor_att, proj (tiny).
  P4  (per 8-image-row group g of 448 cols): per head: depthwise conv as
      per-tap diagonal matmuls accumulating in PSUM (PE); q^T chunk (PE);
      factor = kv_h^T-free matmul (PE); att = factor + q*conv (DVE);
      then proj: out^T[e, cols] = sum_h pw96_h @ att_h (PE) + bias (ACT).

Matmul inputs bf16 (fp32 PSUM accumulation); ~0.5% rel error vs fp32 ref.
"""
import numpy as np
import ml_dtypes

B, N, C = 8, 3137, 768
NH, CH = 8, 96            # heads, channels per head
H = W = 56
HW = H * W                # 3136 == N - 1
NK = 7                    # contraction k-tiles of 128 over 896 (768 + bias row + pad)
KPAD = NK * 128           # 896
HEAD_KS = [3, 3, 5, 5, 5, 7, 7, 7]
SCALE = CH ** -0.5
GROUPS = 7                # image-row groups of 8 rows = 448 cols each
GC = 8 * W                # 448

bf16 = ml_dtypes.bfloat16


def _head_taps(k):
    p = k // 2
    return [(0, 0)] + [(dy, dx) for dy in range(-p, p + 1)
                       for dx in range(-p, p + 1) if (dy, dx) != (0, 0)]


TAPS = [_head_taps(k) for k in HEAD_KS]
TAP_OFF = np.cumsum([0] + [len(t) for t in TAPS]).tolist()  # offsets into dg
NTAP = TAP_OFF[-1]  # 240

_PROG = None


def _build_program():
    import concourse.bass as bass
    from concourse import bacc
    import concourse.mybir as mybir
    import concourse.tile as tile
    from concourse.masks import make_identity
    from contextlib import ExitStack

    fp32 = mybir.dt.float32
    bf = mybir.dt.bfloat16

    nc = bacc.Bacc("TRN2", target_bir_lowering=False, debug=False, num_devices=8)

    xT_d = nc.dram_tensor("xT", [KPAD, N], bf, kind="ExternalInput")
    wq_d = nc.dram_tensor("wq", [KPAD, C], bf, kind="ExternalInput")
    wk_d = nc.dram_tensor("wk", [KPAD, C], bf, kind="ExternalInput")
    wv_d = nc.dram_tensor("wv", [KPAD, C], bf, kind="ExternalInput")
    pw_d = nc.dram_tensor("pw96", [CH, NH, C], bf, kind="ExternalInput")
    pb_d = nc.dram_tensor("pb2", [128, 6], fp32, kind="ExternalInput")
    cb_d = nc.dram_tensor("cb96", [CH, NH], fp32, kind="ExternalInput")
    dg_d = nc.dram_tensor("dg", [CH, NTAP, CH], bf, kind="ExternalInput")
    out_d = nc.dram_tensor("outT", [C, N], fp32, kind="ExternalOutput")

    xT_r = xT_d[:, :].rearrange("(t p) n -> p t n", p=128)  # [128, 7, N]
    wq_r = wq_d[:, :].rearrange("(t p) c -> p t c", p=128)
    wk_r = wk_d[:, :].rearrange("(t p) c -> p t c", p=128)
    wv_r = wv_d[:, :].rearrange("(t p) c -> p t c", p=128)

    # token chunks for pass 1: cls + 24 full + tail(64), img-aligned after cls
    chunks = [(0, 1)] + [(1 + 128 * t, 128) for t in range(24)] + [(3073, 64)]

    with tile.TileContext(nc) as tc, ExitStack() as ctx:
        const = ctx.enter_context(tc.tile_pool(name="const", bufs=1))
        wq_sb = const.tile([128, NK, C], bf)
        nc.sync.dma_start(wq_sb, wq_r)
        pw_sb = const.tile([CH, NH, C], bf)
        nc.sync.dma_start(pw_sb, pw_d[:, :, :])
        pb_sb = const.tile([128, 6], fp32)
        nc.sync.dma_start(pb_sb, pb_d[:, :])
        cb_sb = const.tile([CH, NH], fp32)
        nc.sync.dma_start(cb_sb, cb_d[:, :])
        dg_sb = const.tile([CH, NTAP, CH], bf)
        nc.sync.dma_start(dg_sb, dg_d[:, :, :])
        ident = const.tile([128, 128], bf)
        make_identity(nc, ident)
        ones = const.tile([128, 1], bf)
        nc.vector.memset(ones, 1.0)
        vt = [const.tile([CH, HW], bf, tag=f"vt{h}", name=f"vt{h}") for h in range(NH)]
        kv_sb = const.tile([CH, NH, CH], bf)
        r_sb = const.tile([CH, NH], fp32)

        # ---------------- pass 1: k, v, E, kv accumulation, v transposes
        with tc.tile_pool(name="p1w", bufs=1) as p1w, \
             tc.tile_pool(name="p1roll", bufs=3) as p1roll, \
             tc.tile_pool(name="p1ps", bufs=2, space="PSUM") as p1ps, \
             tc.tile_pool(name="kvps", bufs=1, space="PSUM") as kvps:
            wk_sb = p1w.tile([128, NK, C], bf)
            nc.sync.dma_start(wk_sb, wk_r)
            wv_sb = p1w.tile([128, NK, C], bf)
            nc.sync.dma_start(wv_sb, wv_r)

            kv_ps = [kvps.tile([CH, 4, CH + 1], fp32, tag=f"kv{i}", name=f"kvps{i}") for i in range(2)]

            for ci, (n0, sz) in enumerate(chunks):
                first, last = ci == 0, ci == len(chunks) - 1
                xh = p1roll.tile([128, NK, 128], bf, tag="xh")
                nc.sync.dma_start(xh[:, :, :sz], xT_r[:, :, n0:n0 + sz])

                v_sb = p1roll.tile([128, C], bf, tag="v")
                e_sb = p1roll.tile([128, C], bf, tag="e")
                for half in range(2):
                    c0 = half * 384
                    pv = p1ps.tile([128, 384], fp32, tag="pv")
                    pk = p1ps.tile([128, 384], fp32, tag="pk")
                    for k in range(NK):
                        nc.tensor.matmul(pv[:sz], xh[:, k, :sz], wv_sb[:, k, c0:c0 + 384],
                                         start=(k == 0), stop=(k == NK - 1))
                    for k in range(NK):
                        nc.tensor.matmul(pk[:sz], xh[:, k, :sz], wk_sb[:, k, c0:c0 + 384],
                                         start=(k == 0), stop=(k == NK - 1))
                    nc.scalar.copy(v_sb[:sz, c0:c0 + 384], pv[:sz])
                    nc.scalar.activation(e_sb[:sz, c0:c0 + 384], pk[:sz],
                                         mybir.ActivationFunctionType.Exp)

                for h in range(NH):
                    kvp = kv_ps[h // 4]
                    nc.tensor.matmul(kvp[:, h % 4, 0:CH],
                                     e_sb[:sz, h * CH:(h + 1) * CH],
                                     v_sb[:sz, h * CH:(h + 1) * CH],
                                     start=first, stop=last, skip_group_check=True)
                    nc.tensor.matmul(kvp[:, h % 4, CH:CH + 1],
                                     e_sb[:sz, h * CH:(h + 1) * CH],
                                     ones[:sz],
                                     start=first, stop=last, skip_group_check=True)

                if not first:  # transpose image tokens into vt[h]
                    j0 = n0 - 1
                    for h in range(NH):
                        tp = p1ps.tile([CH, 128], bf, tag="tp")
                        nc.tensor.transpose(tp[:, :sz],
                                            v_sb[:sz, h * CH:(h + 1) * CH],
                                            ident[:sz, :sz])
                        if h % 2 == 0:
                            nc.vector.tensor_copy(vt[h][:, j0:j0 + sz], tp[:, :sz])
                        else:
                            nc.scalar.copy(vt[h][:, j0:j0 + sz], tp[:, :sz])

            # finalize kv: kv = kv_aug[:, :96] * (1/den) * scale
            for h in range(NH):
                kvp = kv_ps[h // 4]
                nc.vector.reciprocal(r_sb[:, h:h + 1], kvp[:, h % 4, CH:CH + 1])
                nc.vector.tensor_scalar(kv_sb[:, h, :], kvp[:, h % 4, 0:CH],
                                        r_sb[:, h:h + 1], float(SCALE),
                                        op0=mybir.AluOpType.mult,
                                        op1=mybir.AluOpType.mult)

        # ---------------- pass 2: per-group conv + q + factor + att + proj
        with tc.tile_pool(name="p4roll", bufs=3) as p4roll, \
             tc.tile_pool(name="p4att", bufs=2) as p4att, \
             tc.tile_pool(name="p4xg", bufs=2) as p4xg, \
             tc.tile_pool(name="p4ps", bufs=2, space="PSUM") as p4ps:

            # cls column (token 0): factor_att only, crpe = 0
            xc = p4xg.tile([128, NK, GC], bf, tag="xg")
            nc.sync.dma_start(xc[:, :, 0:1], xT_r[:, :, 0:1])
            pqc = p4ps.tile([CH, NH], fp32, tag="pq")
            for h in range(NH):
                for k in range(NK):
                    nc.tensor.matmul(pqc[:, h:h + 1], wq_sb[:, k, h * CH:(h + 1) * CH],
                                     xc[:, k, 0:1], start=(k == 0), stop=(k == NK - 1),
                                     skip_group_check=True)
            qtc = p4roll.tile([CH, NH], bf, tag="qtc")
            nc.scalar.copy(qtc, pqc)
            pfc = p4ps.tile([CH, NH], fp32, tag="pf")
            for h in range(NH):
                nc.tensor.matmul(pfc[:, h:h + 1], kv_sb[:, h, :], qtc[:, h:h + 1],
                                 start=True, stop=True, skip_group_check=True)
            atc = p4roll.tile([CH, NH], bf, tag="atc")
            nc.scalar.copy(atc, pfc)
            poc = p4ps.tile([128, GC], fp32, tag="po")
            for e in range(6):
                for h in range(NH):
                    nc.tensor.matmul(poc[:, e:e + 1], pw_sb[:, h, e * 128:(e + 1) * 128],
                                     atc[:, h:h + 1], start=(h == 0), stop=(h == NH - 1),
                                     skip_group_check=True)
            ocs = p4roll.tile([128, 6], fp32, tag="ocs")
            for e in range(6):
                nc.scalar.activation(ocs[:, e:e + 1], poc[:, e:e + 1],
                                     mybir.ActivationFunctionType.Identity,
                                     bias=pb_sb[:, e:e + 1])
                nc.sync.dma_start(out_d[e * 128:(e + 1) * 128, 0:1], ocs[:, e:e + 1])

            # main grouped loop
            vt3 = [vt[h].rearrange("p (y x) -> p y x", y=H) for h in range(NH)]
            for g in range(GROUPS):
                gy0, gy1 = g * 8, g * 8 + 8
                n0 = 1 + g * GC  # token index of first col in group
                xg = p4xg.tile([128, NK, GC], bf, tag="xg")
                nc.sync.dma_start(xg, xT_r[:, :, n0:n0 + GC])

                att = p4att.tile([CH, NH, GC], bf, tag="att")
                for h in range(NH):
                    # conv: per-tap diagonal matmuls accumulating in psum
                    pcv = p4ps.tile([CH, 8, W], fp32, tag="pcv")
                    taps = TAPS[h]
                    t_base = TAP_OFF[h]
                    # which taps actually hit this group
                    live = []
                    for t, (dy, dx) in enumerate(taps):
                        y0 = max(gy0, -dy)
                        y1 = min(gy1, H - max(0, dy))
                        if y1 > y0:
                            live.append((t, dy, dx, y0, y1))
                    assert live[0][0] == 0  # (0,0) full-range first
                    for li, (t, dy, dx, y0, y1) in enumerate(live):
                        x0 = max(0, -dx)
                        x1 = W - max(0, dx)
                        out_ap = pcv[:, y0 - gy0:y1 - gy0, x0:x1]
                        in_ap = vt3[h][:, y0 + dy:y1 + dy, x0 + dx:x1 + dx]
                        nc.tensor.matmul(out_ap, dg_sb[:, t_base + t, :], in_ap,
                                         start=(li == 0), stop=(li == len(live) - 1),
                                         skip_group_check=True)
                    cv = p4roll.tile([CH, GC], bf, tag="cv")
                    nc.scalar.activation(cv, pcv.rearrange("p a b -> p (a b)"),
                                         mybir.ActivationFunctionType.Identity,
                                         bias=cb_sb[:, h:h + 1])

                    # q^T chunk for this head
                    pq = p4ps.tile([CH, GC], fp32, tag="pq")
                    for k in range(NK):
                        nc.tensor.matmul(pq, wq_sb[:, k, h * CH:(h + 1) * CH],
                                         xg[:, k, :], start=(k == 0), stop=(k == NK - 1))
                    qt = p4roll.tile([CH, GC], bf, tag="qt")
                    nc.scalar.copy(qt, pq)

                    # factor_att
                    pf = p4ps.tile([CH, GC], fp32, tag="pf")
                    nc.tensor.matmul(pf, kv_sb[:, h, :], qt, start=True, stop=True)

                    # att = factor + q * conv
                    ev = p4roll.tile([CH, GC], bf, tag="ev")
                    nc.vector.tensor_tensor(ev, qt, cv, op=mybir.AluOpType.mult)
                    nc.vector.scalar_tensor_tensor(att[:, h, :], pf, 1.0, ev,
                                                   op0=mybir.AluOpType.mult,
                                                   op1=mybir.AluOpType.add)

                # proj for this group of columns
                for e in range(6):
                    po = p4ps.tile([128, GC], fp32, tag="po")
                    for h in range(NH):
                        nc.tensor.matmul(po, pw_sb[:, h, e * 128:(e + 1) * 128],
                                         att[:, h, :], start=(h == 0), stop=(h == NH - 1))
                    osb = p4roll.tile([128, GC], fp32, tag="osb")
                    nc.scalar.activation(osb, po,
                                         mybir.ActivationFunctionType.Identity,
                                         bias=pb_sb[:, e:e + 1])
                    nc.sync.dma_start(out_d[e * 128:(e + 1) * 128, n0:n0 + GC], osb)

    nc.compile()
    return nc


def _get_program():
    global _PROG
    if _PROG is None:
        _PROG = _build_program()
    return _PROG


def _host_prep(x, qkv_w, qkv_b, proj_w, proj_b,
               conv3_w, conv3_b, conv5_w, conv5_b, conv7_w, conv7_b):
    """Build per-core input dicts (shared weight tensors prepped once)."""
    qkv_w = np.asarray(qkv_w, np.float32)
    qkv_b = np.asarray(qkv_b, np.float32)
    proj_w = np.asarray(proj_w, np.float32)
    proj_b = np.asarray(proj_b, np.float32)

    def wslab(sl):
        w = np.zeros((KPAD, C), np.float32)
        w[0:C] = qkv_w[sl].T
        w[C] = qkv_b[sl]
        return w.astype(bf16)

    wq = wslab(slice(0, C))
    wk = wslab(slice(C, 2 * C))
    wv = wslab(slice(2 * C, 3 * C))

    pw96 = np.ascontiguousarray(
        proj_w.T.reshape(NH, CH, C).transpose(1, 0, 2)).astype(bf16)
    pb2 = np.ascontiguousarray(proj_b.reshape(6, 128).T).astype(np.float32)

    conv_w = [np.asarray(w, np.float32) for w in (conv3_w, conv5_w, conv7_w)]
    conv_b = [np.asarray(b, np.float32) for b in (conv3_b, conv5_b, conv7_b)]
    grp_of_head = [0, 0, 1, 1, 1, 2, 2, 2]
    head_in_grp = [0, 1, 0, 1, 2, 0, 1, 2]

    cb96 = np.zeros((CH, NH), np.float32)
    dg = np.zeros((CH, NTAP, CH), np.float32)
    for h in range(NH):
        k = HEAD_KS[h]
        p = k // 2
        gidx, hig = grp_of_head[h], head_in_grp[h]
        wfull = conv_w[gidx][hig * CH:(hig + 1) * CH, 0]  # [96, k, k]
        cb96[:, h] = conv_b[gidx][hig * CH:(hig + 1) * CH]
        for t, (dy, dx) in enumerate(TAPS[h]):
            np.fill_diagonal(dg[:, TAP_OFF[h] + t, :], wfull[:, dy + p, dx + p])
    dg = dg.astype(bf16)

    shared = {"wq": wq, "wk": wk, "wv": wv, "pw96": pw96, "pb2": pb2,
              "cb96": cb96, "dg": dg}

    x = np.asarray(x, np.float32)
    in_maps = []
    for b in range(B):
        xT = np.zeros((KPAD, N), np.float32)
        xT[0:C] = x[b].T
        xT[C] = 1.0
        m = dict(shared)
        m["xT"] = xT.astype(bf16)
        in_maps.append(m)
    return in_maps


def kernel(x, qkv_w, qkv_b, proj_w, proj_b,
           conv3_w, conv3_b, conv5_w, conv5_b, conv7_w, conv7_b, H, W,
           _trace=False):
    assert int(H) == 56 and int(W) == 56
    x = np.asarray(x)
    assert x.shape == (B, N, C)

    from concourse.bass_utils import run_bass_kernel_spmd
    nc = _get_program()
    in_maps = _host_prep(x, qkv_w, qkv_b, proj_w, proj_b,
                         conv3_w, conv3_b, conv5_w, conv5_b, conv7_w, conv7_b)
    res = run_bass_kernel_spmd(nc, in_maps, core_ids=list(range(B)), trace=_trace)
    out = np.stack([res.results[b]["outT"].T for b in range(B)])
    if _trace:
        kernel._last_results = res
    return out.astype(np.float32)

